# revision 5
# baseline (speedup 1.0000x reference)
"""GCN message-passing kernel for Trainium2, 8 NeuronCores.

Model (see reference):
    h   = relu(GCNConv(x, edge_index; W1, b1))      # [N, 128]
    p   = segment_max(h, batch, 128 graphs)          # [128, 128]
    out = log_softmax(p @ W2 + b2)                   # [128, 2]

GCNConv with self loops and symmetric norm decomposes as
    out = D^-1/2 * A * D^-1/2 * (x @ W1) + b1
so we compute H' = dinv * (x@W1) row-scaled, aggregate H'[src] into dst with
an unweighted segment-sum, then row-scale by dinv[dst] again.

Distribution (8 cores):
  * nodes row-sharded contiguously: core c owns nodes [c*6250, (c+1)*6250)
  * core c computes its H' slice, then AllGather -> every core holds full H'
  * edges sharded by dst ownership (sorted by dst on host); each core
    aggregates its own dst rows:  for each 128-dst-node tile, gather
    H'[src] rows via indirect DMA, build a one-hot selection matrix from
    the dst indices (iota == dstrel), and matmul-accumulate into PSUM.
  * graph boundaries align with the node shard (6250 nodes = exactly 16
    graphs per core), so segment_max + head are fully core-local.
  * final [16,2] per-core outputs are concatenated on host.
"""

import math

import numpy as np

N_NODES = 50000
N_EDGES = 1600000
N_FEAT = 512
N_HID = 128
N_CLASSES = 2
NUM_GRAPHS = 128
NCORES = 8
NPC = N_NODES // NCORES          # 6250 nodes per core
GPC = NUM_GRAPHS // NCORES       # 16 graphs per core
P = 128
NT = (NPC + P - 1) // P          # 49 dst tiles per core (last tile 106 valid)
KF = N_FEAT // P                 # 4 K-chunks for x @ W1

_PROGRAM_CACHE: dict = {}
LAST_RESULTS = None              # BassKernelResults of the most recent run


def _host_prep(x, W1, b1, W2, b2, edge_index, batch):
    """All integer/index preprocessing + input shard construction."""
    x = np.asarray(x, dtype=np.float32)
    W1 = np.asarray(W1, dtype=np.float32)
    b1 = np.asarray(b1, dtype=np.float32)
    W2 = np.asarray(W2, dtype=np.float32)
    b2 = np.asarray(b2, dtype=np.float32)
    ei = np.asarray(edge_index)
    batch = np.asarray(batch)

    src = ei[0].astype(np.int32)
    dst = ei[1].astype(np.int32)
    loops = np.arange(N_NODES, dtype=np.int32)
    src = np.concatenate([src, loops])
    dst = np.concatenate([dst, loops])

    # in-degree (with self loops); float input to the device rsqrt
    deg = np.bincount(dst, minlength=N_NODES).astype(np.float32)

    order = np.argsort(dst, kind="stable")
    src_s = src[order]
    dst_s = dst[order]

    # graph boundaries from the actual batch tensor; must align to the shard
    gbs = np.searchsorted(batch, np.arange(NUM_GRAPHS + 1))
    gb_local = gbs[:GPC + 1].astype(np.int64).copy()
    for c in range(NCORES):
        seg = gbs[c * GPC:(c + 1) * GPC + 1] - c * NPC
        assert np.array_equal(seg, gb_local), "graph/node shard misalignment"

    # per (core, tile) edge counts -> common chunk schedule across cores
    tile_bounds = np.empty(NCORES * NT + 1, dtype=np.int64)
    k = 0
    for c in range(NCORES):
        for t in range(NT):
            tile_bounds[k] = c * NPC + t * P
            k += 1
    tile_bounds[-1] = N_NODES
    edge_ofs = np.searchsorted(dst_s, tile_bounds)          # [8*49 + 1]
    counts = np.diff(edge_ofs).reshape(NCORES, NT)          # [8, 49]
    chunks = np.maximum(1, -(-counts.max(axis=0) // P))     # [49] ceil-div
    ct0 = np.concatenate([[0], np.cumsum(chunks)])          # [50]
    ctot = int(ct0[-1])

    srci = np.zeros((NCORES, P, ctot), dtype=np.int32)
    dstr = np.full((NCORES, P, ctot), 255.0, dtype=np.float32)
    for c in range(NCORES):
        for t in range(NT):
            e0 = edge_ofs[c * NT + t]
            cnt = counts[c, t]
            if cnt == 0:
                continue
            s = np.arange(cnt)
            pp = s % P
            col = int(ct0[t]) + s // P
            srci[c, pp, col] = src_s[e0:e0 + cnt]
            dstr[c, pp, col] = (dst_s[e0:e0 + cnt]
                                - (c * NPC + t * P)).astype(np.float32)

    # x transposed + padded to the tile grid; deg per-core in [128, NT] layout
    xT = np.ascontiguousarray(x.T)                          # [512, 50000]
    xT_pad = np.zeros((NCORES, N_FEAT, NT * P), dtype=np.float32)
    deg_cols = np.ones((NCORES, P, NT), dtype=np.float32)
    for c in range(NCORES):
        xT_pad[c, :, :NPC] = xT[:, c * NPC:(c + 1) * NPC]
        d = deg[c * NPC:(c + 1) * NPC]                      # [6250]
        dp = np.ones(NT * P, dtype=np.float32)
        dp[:NPC] = d
        deg_cols[c] = dp.reshape(NT, P).T

    iota_mat = np.tile(np.arange(P, dtype=np.float32), (P, 1))
    b1_mat = np.tile(b1[None, :], (P, 1)).astype(np.float32)
    b2_mat = np.tile(b2[None, :], (GPC, 1)).astype(np.float32)

    in_maps = []
    for c in range(NCORES):
        in_maps.append({
            "xT": xT_pad[c],
            "w1": W1,
            "b1m": b1_mat,
            "w2": W2,
            "b2m": b2_mat,
            "degc": deg_cols[c],
            "srci": srci[c],
            "dstr": dstr[c],
            "iot": iota_mat,
        })
    return tuple(int(v) for v in chunks), tuple(int(v) for v in gb_local), in_maps


def _build_program(chunks, gb_local):
    import concourse.bacc as bacc
    import concourse.bass as bass
    import concourse.mybir as mybir
    import concourse.tile as tile
    from concourse.masks import make_identity

    f32 = mybir.dt.float32
    i32 = mybir.dt.int32
    ctot = int(sum(chunks))
    ct0 = [0]
    for cn in chunks:
        ct0.append(ct0[-1] + cn)

    nc = bacc.Bacc("TRN2", target_bir_lowering=False, debug=False,
                   num_devices=NCORES)

    xT = nc.dram_tensor("xT", [N_FEAT, NT * P], f32, kind="ExternalInput")
    w1 = nc.dram_tensor("w1", [N_FEAT, N_HID], f32, kind="ExternalInput")
    b1m = nc.dram_tensor("b1m", [P, N_HID], f32, kind="ExternalInput")
    w2 = nc.dram_tensor("w2", [N_HID, N_CLASSES], f32, kind="ExternalInput")
    b2m = nc.dram_tensor("b2m", [GPC, N_CLASSES], f32, kind="ExternalInput")
    degc = nc.dram_tensor("degc", [P, NT], f32, kind="ExternalInput")
    srci = nc.dram_tensor("srci", [P, ctot], i32, kind="ExternalInput")
    dstr = nc.dram_tensor("dstr", [P, ctot], f32, kind="ExternalInput")
    iot = nc.dram_tensor("iot", [P, P], f32, kind="ExternalInput")
    out = nc.dram_tensor("out", [GPC, N_CLASSES], f32, kind="ExternalOutput")

    with tile.TileContext(nc) as tc:
        with (
            tc.tile_pool(name="const", bufs=1) as constp,
            tc.tile_pool(name="xt", bufs=3) as xtp,
            tc.tile_pool(name="hp", bufs=3) as hpp,
            tc.tile_pool(name="oh", bufs=2) as ohp,
            tc.tile_pool(name="gg", bufs=2) as ggp,
            tc.tile_pool(name="hr", bufs=3) as hrp,
            tc.tile_pool(name="ps_h", bufs=2, space="PSUM") as psh,
            tc.tile_pool(name="ps_a", bufs=2, space="PSUM") as psa,
            tc.tile_pool(name="ps_t", bufs=2, space="PSUM") as pst,
            tc.tile_pool(name="dram", bufs=1, space="DRAM") as dramp,
        ):
            # ---- constants / persistent state ----
            w1_sb = constp.tile([P, N_FEAT], f32)            # [p, (k n)]
            nc.sync.dma_start(
                out=w1_sb[:].rearrange("p (k n) -> p k n", k=KF),
                in_=w1.ap().rearrange("(k p) n -> p k n", p=P))
            iota_sb = constp.tile([P, P], f32)
            nc.sync.dma_start(out=iota_sb[:], in_=iot.ap())
            b1_sb = constp.tile([P, N_HID], f32)
            nc.sync.dma_start(out=b1_sb[:], in_=b1m.ap())
            w2_sb = constp.tile([N_HID, N_CLASSES], f32)
            nc.sync.dma_start(out=w2_sb[:], in_=w2.ap())
            b2_sb = constp.tile([GPC, N_CLASSES], f32)
            nc.sync.dma_start(out=b2_sb[:], in_=b2m.ap())
            ident = constp.tile([P, P], f32)
            make_identity(nc, ident[:])

            deg_sb = constp.tile([P, NT], f32)
            nc.sync.dma_start(out=deg_sb[:], in_=degc.ap())
            rec_sb = constp.tile([P, NT], f32)
            nc.vector.reciprocal(rec_sb[:], deg_sb[:])
            dinv_sb = constp.tile([P, NT], f32)
            nc.scalar.activation(dinv_sb[:], rec_sb[:],
                                 mybir.ActivationFunctionType.Sqrt)

            srci_sb = constp.tile([P, ctot], i32)
            nc.sync.dma_start(out=srci_sb[:], in_=srci.ap())
            dstr_sb = constp.tile([P, ctot], f32)
            nc.sync.dma_start(out=dstr_sb[:], in_=dstr.ap())

            hT_sb = constp.tile([P, NT * P], f32)            # [hid, node]

            cc_in = dramp.tile([NPC, N_HID], f32)
            cc_full = dramp.tile([N_NODES, N_HID], f32, addr_space="Shared")

            # ---- phase A: H' = dinv * (x @ W1), per 128-node tile ----
            for t in range(NT):
                xt_t = xtp.tile([P, N_FEAT], f32, tag="xt")
                nc.sync.dma_start(
                    out=xt_t[:].rearrange("p (k m) -> p k m", k=KF),
                    in_=xT.ap()[:, t * P:(t + 1) * P]
                        .rearrange("(k p) m -> p k m", p=P))
                ph = psh.tile([P, N_HID], f32, tag="ph")
                for k in range(KF):
                    nc.tensor.matmul(ph[:],
                                     lhsT=xt_t[:, k * P:(k + 1) * P],
                                     rhs=w1_sb[:, k * P:(k + 1) * P],
                                     start=(k == 0), stop=(k == KF - 1))
                hp_t = hpp.tile([P, N_HID], f32, tag="hp")
                nc.vector.tensor_scalar_mul(hp_t[:], ph[:],
                                            dinv_sb[:, t:t + 1])
                lo = t * P
                hi = min((t + 1) * P, NPC)
                nc.sync.dma_start(out=cc_in[lo:hi, :],
                                  in_=hp_t[:hi - lo, :])

            # ---- all-gather H' across the 8 cores ----
            nc.gpsimd.collective_compute(
                "AllGather", mybir.AluOpType.bypass,
                replica_groups=[list(range(NCORES))],
                ins=[cc_in[:, :]],
                outs=[cc_full[:, :]],
            )

            # ---- phase B: aggregate per dst tile ----
            for t in range(NT):
                cn = chunks[t]
                g_t = ggp.tile([P, cn * P], f32, tag="g")
                oh_t = ohp.tile([P, cn * P], f32, tag="oh")
                for j in range(cn):
                    col = ct0[t] + j
                    nc.gpsimd.indirect_dma_start(
                        out=g_t[:, j * P:(j + 1) * P],
                        out_offset=None,
                        in_=cc_full[:, :],
                        in_offset=bass.IndirectOffsetOnAxis(
                            ap=srci_sb[:, col:col + 1], axis=0),
                    )
                # one-hot: oh[p, (c n)] = (iota[n] == dstrel[p, c])
                nc.vector.tensor_tensor(
                    out=oh_t[:].rearrange("p (c n) -> p c n", n=P),
                    in0=iota_sb[:].rearrange("p (one n) -> p one n", one=1)
                        .to_broadcast([P, cn, P]),
                    in1=dstr_sb[:, ct0[t]:ct0[t] + cn]
                        .rearrange("p (c one) -> p c one", one=1)
                        .to_broadcast([P, cn, P]),
                    op=mybir.AluOpType.is_equal,
                )
                pa = psa.tile([P, N_HID], f32, tag="pa")
                for j in range(cn):
                    nc.tensor.matmul(pa[:],
                                     lhsT=oh_t[:, j * P:(j + 1) * P],
                                     rhs=g_t[:, j * P:(j + 1) * P],
                                     start=(j == 0), stop=(j == cn - 1))
                hr = hrp.tile([P, N_HID], f32, tag="hr")
                nc.vector.tensor_scalar_mul(hr[:], pa[:], dinv_sb[:, t:t + 1])
                nc.vector.tensor_add(out=hr[:], in0=hr[:], in1=b1_sb[:])
                nc.vector.tensor_scalar_max(hr[:], hr[:], 0.0)
                ptr = pst.tile([P, P], f32, tag="pt")
                nc.tensor.transpose(out=ptr[:], in_=hr[:], identity=ident[:])
                nc.vector.tensor_copy(out=hT_sb[:, t * P:(t + 1) * P],
                                      in_=ptr[:])

            # ---- phase C: segment max pooling + linear head + log_softmax ----
            pooled = constp.tile([P, GPC], f32)
            for k in range(GPC):
                nc.vector.reduce_max(pooled[:, k:k + 1],
                                     hT_sb[:, gb_local[k]:gb_local[k + 1]],
                                     axis=mybir.AxisListType.X)
            pl = psa.tile([GPC, N_CLASSES], f32, tag="pl")
            nc.tensor.matmul(pl[:], lhsT=pooled[:], rhs=w2_sb[:],
                             start=True, stop=True)
            ls = constp.tile([GPC, N_CLASSES], f32)
            nc.vector.tensor_add(out=ls[:], in0=pl[:], in1=b2_sb[:])
            mx = constp.tile([GPC, 1], f32)
            nc.vector.reduce_max(mx[:], ls[:], axis=mybir.AxisListType.X)
            xm = constp.tile([GPC, N_CLASSES], f32)
            nc.vector.tensor_scalar_sub(xm[:], ls[:], mx[:, 0:1])
            ex = constp.tile([GPC, N_CLASSES], f32)
            nc.scalar.activation(ex[:], xm[:],
                                 mybir.ActivationFunctionType.Exp)
            sm = constp.tile([GPC, 1], f32)
            nc.vector.reduce_sum(sm[:], ex[:], axis=mybir.AxisListType.X)
            lg = constp.tile([GPC, 1], f32)
            nc.scalar.activation(lg[:], sm[:],
                                 mybir.ActivationFunctionType.Ln)
            of = constp.tile([GPC, N_CLASSES], f32)
            nc.vector.tensor_scalar_sub(of[:], xm[:], lg[:, 0:1])
            nc.sync.dma_start(out=out.ap()[:, :], in_=of[:])

    nc.compile()
    return nc


def _install_ntff_hook():
    """Provide antenv.axon_hooks.get_axon_ntff_profile_hook when the agent
    image lacks it (ctypes bridge into libaxon_pjrt.so, mirroring
    trn_boot._ntff_profile_via_ctypes)."""
    import contextlib
    import ctypes
    import sys
    import types
    try:
        import antenv.axon_hooks  # noqa: F401
        return
    except ImportError:
        pass
    the_hook = None
    try:
        lib = ctypes.CDLL("/opt/axon/libaxon_pjrt.so")
        if hasattr(lib, "axon_start_nrt_profile"):
            lib.axon_start_nrt_profile.argtypes = [
                ctypes.POINTER(ctypes.c_int64), ctypes.c_size_t]
            lib.axon_start_nrt_profile.restype = ctypes.c_int64
            lib.axon_stop_nrt_profile.argtypes = [ctypes.c_char_p]
            lib.axon_stop_nrt_profile.restype = ctypes.c_int64

            @contextlib.contextmanager
            def _hook(output_dir, device_ids):
                import jax
                jax.devices()
                if device_ids:
                    ids = (ctypes.c_int64 * len(device_ids))(*device_ids)
                    rc = lib.axon_start_nrt_profile(ids, len(device_ids))
                else:
                    rc = lib.axon_start_nrt_profile(None, 0)
                if rc != 0:
                    raise RuntimeError(f"axon_start_nrt_profile rc={rc}")
                try:
                    yield
                finally:
                    n = lib.axon_stop_nrt_profile(str(output_dir).encode())
                    print(f"ntff profile: {n} file(s) -> {output_dir}")

            the_hook = _hook
    except OSError:
        pass
    mod = types.ModuleType("antenv.axon_hooks")
    mod.get_axon_ntff_profile_hook = lambda: the_hook
    mod.set_axon_ntff_profile_hook = lambda h: None
    import antenv
    antenv.axon_hooks = mod
    sys.modules["antenv.axon_hooks"] = mod


def kernel(x, W1, b1, W2, b2, edge_index, batch, _trace=False, _trace_kwargs=None):
    global LAST_RESULTS
    from concourse import bass_utils
    if _trace:
        _install_ntff_hook()
        # no fish/S3 in this container; keep artifacts local
        bass_utils.upload_artifacts = lambda tmpdir: tmpdir

    chunks, gb_local, in_maps = _host_prep(x, W1, b1, W2, b2,
                                           edge_index, batch)
    key = (chunks, gb_local)
    nc = _PROGRAM_CACHE.get(key)
    if nc is None:
        nc = _build_program(chunks, gb_local)
        _PROGRAM_CACHE[key] = nc

    res = bass_utils.run_bass_kernel_spmd(
        nc, in_maps, core_ids=list(range(NCORES)),
        trace=_trace, tmpdir="/tmp/gnn_neff" if _trace else None,
        **(_trace_kwargs or {}))
    LAST_RESULTS = res
    return np.concatenate([res.results[c]["out"] for c in range(NCORES)],
                          axis=0)


# revision 16
# speedup vs baseline: 2.6544x; 2.6544x over previous
"""GCN message-passing kernel for Trainium2, 8 NeuronCores.

Model (see reference):
    h   = relu(GCNConv(x, edge_index; W1, b1))      # [N, 128]
    p   = segment_max(h, batch, 128 graphs)          # [128, 128]
    out = log_softmax(p @ W2 + b2)                   # [128, 2]

GCNConv with self loops and symmetric norm decomposes as
    out = D^-1/2 * A * D^-1/2 * (x @ W1) + b1
so we compute H' = dinv * (x@W1) row-scaled, aggregate H'[src] into dst with
an unweighted segment-sum, then row-scale by dinv[dst] again.

Distribution (8 cores):
  * nodes row-sharded contiguously: core c owns nodes [c*6250, (c+1)*6250)
  * core c computes its H' slice, then AllGather -> every core holds full H'
  * edges sharded by dst ownership (sorted by dst on host); each core
    aggregates its own dst rows:  for each 128-dst-node tile, gather
    H'[src] rows via indirect DMA, build a one-hot selection matrix from
    the dst indices (iota == dstrel), and matmul-accumulate into PSUM.
  * graph boundaries align with the node shard (6250 nodes = exactly 16
    graphs per core), so segment_max + head are fully core-local.
  * final [16,2] per-core outputs are concatenated on host.
"""

import math

import numpy as np

N_NODES = 50000
N_EDGES = 1600000
N_FEAT = 512
N_HID = 128
N_CLASSES = 2
NUM_GRAPHS = 128
NCORES = 8
NPC = N_NODES // NCORES          # 6250 nodes per core
GPC = NUM_GRAPHS // NCORES       # 16 graphs per core
P = 128
NT = (NPC + P - 1) // P          # 49 dst tiles per core (last tile 106 valid)
KF = N_FEAT // P                 # 4 K-chunks for x @ W1
HALF = N_NODES // 2              # dma_gather idx is int16 -> split table

_PROGRAM_CACHE: dict = {}
LAST_RESULTS = None              # BassKernelResults of the most recent run


def _host_prep(x, W1, b1, W2, b2, edge_index, batch):
    """All integer/index preprocessing + input shard construction."""
    x = np.asarray(x, dtype=np.float32)
    W1 = np.asarray(W1, dtype=np.float32)
    b1 = np.asarray(b1, dtype=np.float32)
    W2 = np.asarray(W2, dtype=np.float32)
    b2 = np.asarray(b2, dtype=np.float32)
    ei = np.asarray(edge_index)
    batch = np.asarray(batch)

    src = ei[0].astype(np.int32)
    dst = ei[1].astype(np.int32)
    loops = np.arange(N_NODES, dtype=np.int32)
    src = np.concatenate([src, loops])
    dst = np.concatenate([dst, loops])

    # in-degree (with self loops); float input to the device rsqrt
    deg = np.bincount(dst, minlength=N_NODES).astype(np.float32)

    # sort edges by (dst tile, src half).  The src half is a needed sort key
    # because dma_gather indices are int16 -> the gather table splits at HALF;
    # dst order within a tile is irrelevant (one-hot handles it per edge).
    half_of = (src >= HALF).astype(np.int64)
    tid = (dst // NPC) * NT + (dst % NPC) // P               # global dst tile
    order = np.lexsort((half_of, tid))
    src_s = src[order]
    dst_s = dst[order]
    half_s = half_of[order]
    tid_s = tid[order]

    # graph boundaries from the actual batch tensor; must align to the shard
    gbs = np.searchsorted(batch, np.arange(NUM_GRAPHS + 1))
    gb_local = gbs[:GPC + 1].astype(np.int64).copy()
    for c in range(NCORES):
        seg = gbs[c * GPC:(c + 1) * GPC + 1] - c * NPC
        assert np.array_equal(seg, gb_local), "graph/node shard misalignment"

    # per (core, tile, half) edge counts -> common chunk schedule across cores
    edge_ofs = np.searchsorted(tid_s, np.arange(NCORES * NT + 1))
    # boundary between halves within each (c, t) run
    half_break = np.empty(NCORES * NT, dtype=np.int64)
    for i in range(NCORES * NT):
        e0, e1 = edge_ofs[i], edge_ofs[i + 1]
        half_break[i] = e0 + np.searchsorted(half_s[e0:e1], 1)
    cnt_h = np.stack([half_break - edge_ofs[:-1],
                      edge_ofs[1:] - half_break])            # [2, 8*49]
    cnt_h = cnt_h.reshape(2, NCORES, NT)
    # chunks per (tile, half), shared across cores (SPMD: one program)
    chunks = -(-cnt_h.max(axis=1).T // P)                    # [NT, 2] ceil
    ct0 = np.zeros((NT, 2), dtype=np.int64)                  # chunk col offset
    q0 = np.zeros((NT, 2), dtype=np.int64)                   # idx col16 offset
    acc_c = acc_q = 0
    for t in range(NT):
        for h in range(2):
            ct0[t, h] = acc_c
            q0[t, h] = acc_q
            acc_c += chunks[t, h]
            acc_q += chunks[t, h] * 8                        # 128/16 cols16
    ctot = int(acc_c)
    qtot = int(acc_q)

    gidx = np.zeros((NCORES, P, qtot), dtype=np.int16)
    dstr = np.full((NCORES, P, ctot), 255.0, dtype=np.float32)
    for c in range(NCORES):
        for t in range(NT):
            for h in range(2):
                if h == 0:
                    e0 = edge_ofs[c * NT + t]
                    cnt = int(cnt_h[0, c, t])
                else:
                    e0 = half_break[c * NT + t]
                    cnt = int(cnt_h[1, c, t])
                K = int(chunks[t, h]) * P                    # padded len
                if K == 0:
                    continue
                idx = np.zeros(K, dtype=np.int16)            # pad: row 0
                idx[:cnt] = (src_s[e0:e0 + cnt] - h * HALF).astype(np.int16)
                # wrapped in 16 partitions, replicated for all 8 Q7 cores
                gidx[c, :, q0[t, h]:q0[t, h] + K // 16] = \
                    np.tile(idx.reshape(K // 16, 16).T, (8, 1))
                if cnt:
                    s = np.arange(cnt)
                    dstr[c, s % P, ct0[t, h] + s // P] = \
                        (dst_s[e0:e0 + cnt]
                         - (c * NPC + t * P)).astype(np.float32)

    # x transposed + padded to the tile grid; deg per-core in [128, NT] layout
    xT = np.ascontiguousarray(x.T)                          # [512, 50000]
    xT_pad = np.zeros((NCORES, N_FEAT, NT * P), dtype=np.float32)
    deg_cols = np.ones((NCORES, P, NT), dtype=np.float32)
    for c in range(NCORES):
        xT_pad[c, :, :NPC] = xT[:, c * NPC:(c + 1) * NPC]
        d = deg[c * NPC:(c + 1) * NPC]                      # [6250]
        dp = np.ones(NT * P, dtype=np.float32)
        dp[:NPC] = d
        deg_cols[c] = dp.reshape(NT, P).T

    iota_mat = np.tile(np.arange(P, dtype=np.float32), (P, 1))
    b1_mat = np.tile(b1[None, :], (P, 1)).astype(np.float32)
    b2_mat = np.tile(b2[None, :], (GPC, 1)).astype(np.float32)

    in_maps = []
    for c in range(NCORES):
        in_maps.append({
            "xT": xT_pad[c],
            "w1": W1,
            "b1m": b1_mat,
            "w2": W2,
            "b2m": b2_mat,
            "degc": deg_cols[c],
            "gidx": gidx[c],
            "dstr": dstr[c],
            "iot": iota_mat,
        })
    chunks_key = tuple(tuple(int(v) for v in row) for row in chunks)
    return chunks_key, tuple(int(v) for v in gb_local), in_maps


def _build_program(chunks, gb_local):
    import concourse.bacc as bacc
    import concourse.bass as bass
    import concourse.mybir as mybir
    import concourse.tile as tile
    from concourse.masks import make_identity

    f32 = mybir.dt.float32
    f16 = mybir.dt.float16
    i16 = mybir.dt.int16
    # chunks: [NT][2] chunk counts; column/idx offsets mirror _host_prep
    ct0 = [[0, 0] for _ in range(NT)]
    q0 = [[0, 0] for _ in range(NT)]
    acc_c = acc_q = 0
    for t in range(NT):
        for h in range(2):
            ct0[t][h] = acc_c
            q0[t][h] = acc_q
            acc_c += chunks[t][h]
            acc_q += chunks[t][h] * 8
    ctot = acc_c
    qtot = acc_q

    nc = bacc.Bacc("TRN2", target_bir_lowering=False, debug=False,
                   num_devices=NCORES, num_swdge_queues=4)

    xT = nc.dram_tensor("xT", [N_FEAT, NT * P], f32, kind="ExternalInput")
    w1 = nc.dram_tensor("w1", [N_FEAT, N_HID], f32, kind="ExternalInput")
    b1m = nc.dram_tensor("b1m", [P, N_HID], f32, kind="ExternalInput")
    w2 = nc.dram_tensor("w2", [N_HID, N_CLASSES], f32, kind="ExternalInput")
    b2m = nc.dram_tensor("b2m", [GPC, N_CLASSES], f32, kind="ExternalInput")
    degc = nc.dram_tensor("degc", [P, NT], f32, kind="ExternalInput")
    gidx = nc.dram_tensor("gidx", [P, qtot], i16, kind="ExternalInput")
    dstr = nc.dram_tensor("dstr", [P, ctot], f32, kind="ExternalInput")
    iot = nc.dram_tensor("iot", [P, P], f32, kind="ExternalInput")
    out = nc.dram_tensor("out", [GPC, N_CLASSES], f32, kind="ExternalOutput")

    with tile.TileContext(nc) as tc:
        with (
            tc.tile_pool(name="const", bufs=1) as constp,
            tc.tile_pool(name="xt", bufs=3) as xtp,
            tc.tile_pool(name="hp", bufs=3) as hpp,
            tc.tile_pool(name="oh", bufs=2) as ohp,
            tc.tile_pool(name="gg", bufs=2) as ggp,
            tc.tile_pool(name="hr", bufs=3) as hrp,
            tc.tile_pool(name="ps_h", bufs=2, space="PSUM") as psh,
            tc.tile_pool(name="ps_a", bufs=2, space="PSUM") as psa,
            tc.tile_pool(name="ps_t", bufs=2, space="PSUM") as pst,
            tc.tile_pool(name="dram", bufs=1, space="DRAM") as dramp,
        ):
            # ---- constants / persistent state ----
            w1_sb = constp.tile([P, N_FEAT], f32)            # [p, (k n)]
            nc.sync.dma_start(
                out=w1_sb[:].rearrange("p (k n) -> p k n", k=KF),
                in_=w1.ap().rearrange("(k p) n -> p k n", p=P))
            iota_sb = constp.tile([P, P], f32)
            nc.sync.dma_start(out=iota_sb[:], in_=iot.ap())
            b1_sb = constp.tile([P, N_HID], f32)
            nc.sync.dma_start(out=b1_sb[:], in_=b1m.ap())
            w2_sb = constp.tile([N_HID, N_CLASSES], f32)
            nc.sync.dma_start(out=w2_sb[:], in_=w2.ap())
            b2_sb = constp.tile([GPC, N_CLASSES], f32)
            nc.sync.dma_start(out=b2_sb[:], in_=b2m.ap())
            ident = constp.tile([P, P], f32)
            make_identity(nc, ident[:])

            deg_sb = constp.tile([P, NT], f32)
            nc.sync.dma_start(out=deg_sb[:], in_=degc.ap())
            rec_sb = constp.tile([P, NT], f32)
            nc.vector.reciprocal(rec_sb[:], deg_sb[:])
            dinv_sb = constp.tile([P, NT], f32)
            nc.scalar.activation(dinv_sb[:], rec_sb[:],
                                 mybir.ActivationFunctionType.Sqrt)

            gidx_sb = constp.tile([P, qtot], i16)
            nc.sync.dma_start(out=gidx_sb[:], in_=gidx.ap())
            dstr_sb = constp.tile([P, ctot], f32)
            nc.sync.dma_start(out=dstr_sb[:], in_=dstr.ap())

            hT_sb = constp.tile([P, NT * P], f32)            # [hid, node]

            cc_in = dramp.tile([NPC, N_HID], f16)
            cc_full = dramp.tile([N_NODES, N_HID], f16, addr_space="Shared")

            # ---- phase A: H' = dinv * (x @ W1), per 128-node tile ----
            for t in range(NT):
                xt_t = xtp.tile([P, N_FEAT], f32, tag="xt")
                nc.sync.dma_start(
                    out=xt_t[:].rearrange("p (k m) -> p k m", k=KF),
                    in_=xT.ap()[:, t * P:(t + 1) * P]
                        .rearrange("(k p) m -> p k m", p=P))
                ph = psh.tile([P, N_HID], f32, tag="ph")
                for k in range(KF):
                    nc.tensor.matmul(ph[:],
                                     lhsT=xt_t[:, k * P:(k + 1) * P],
                                     rhs=w1_sb[:, k * P:(k + 1) * P],
                                     start=(k == 0), stop=(k == KF - 1))
                hp_t = hpp.tile([P, N_HID], f16, tag="hp")
                nc.vector.tensor_scalar_mul(hp_t[:], ph[:],
                                            dinv_sb[:, t:t + 1])
                lo = t * P
                hi = min((t + 1) * P, NPC)
                nc.sync.dma_start(out=cc_in[lo:hi, :],
                                  in_=hp_t[:hi - lo, :])

            # ---- all-gather H' across the 8 cores ----
            nc.gpsimd.collective_compute(
                "AllGather", mybir.AluOpType.bypass,
                replica_groups=[list(range(NCORES))],
                ins=[cc_in[:, :]],
                outs=[cc_full[:, :]],
            )

            # ---- phase B: aggregate per dst tile ----
            qn = 0
            for t in range(NT):
                cn = chunks[t][0] + chunks[t][1]
                g_t = ggp.tile([P, cn * P], f16, tag="g")
                oh_t = ohp.tile([P, cn * P], f16, tag="oh")
                goff = 0
                for h in range(2):
                    ch = chunks[t][h]
                    if ch == 0:
                        continue
                    K = ch * P
                    nc.gpsimd.dma_gather(
                        out_ap=g_t[:, goff * P:(goff + ch) * P]
                            .rearrange("p (j f) -> p j f", f=N_HID),
                        in_ap=cc_full[h * HALF:(h + 1) * HALF, :],
                        idxs_ap=gidx_sb[:, q0[t][h]:q0[t][h] + K // 16],
                        num_idxs=K,
                        num_idxs_reg=K,
                        elem_size=N_HID,
                        single_packet=K <= 1024,
                        queue_num=qn % 4,
                    )
                    qn += 1
                    goff += ch
                # one-hot: oh[p, (c n)] = (iota[n] == dstrel[p, c])
                nc.vector.tensor_tensor(
                    out=oh_t[:].rearrange("p (c n) -> p c n", n=P),
                    in0=iota_sb[:].rearrange("p (one n) -> p one n", one=1)
                        .to_broadcast([P, cn, P]),
                    in1=dstr_sb[:, ct0[t][0]:ct0[t][0] + cn]
                        .rearrange("p (c one) -> p c one", one=1)
                        .to_broadcast([P, cn, P]),
                    op=mybir.AluOpType.is_equal,
                )
                pa = psa.tile([P, N_HID], f32, tag="pa")
                for j in range(cn):
                    nc.tensor.matmul(pa[:],
                                     lhsT=oh_t[:, j * P:(j + 1) * P],
                                     rhs=g_t[:, j * P:(j + 1) * P],
                                     start=(j == 0), stop=(j == cn - 1))
                hr = hrp.tile([P, N_HID], f32, tag="hr")
                nc.vector.tensor_scalar_mul(hr[:], pa[:], dinv_sb[:, t:t + 1])
                nc.vector.tensor_add(out=hr[:], in0=hr[:], in1=b1_sb[:])
                nc.vector.tensor_scalar_max(hr[:], hr[:], 0.0)
                ptr = pst.tile([P, P], f32, tag="pt")
                nc.tensor.transpose(out=ptr[:], in_=hr[:], identity=ident[:])
                nc.vector.tensor_copy(out=hT_sb[:, t * P:(t + 1) * P],
                                      in_=ptr[:])

            # ---- phase C: segment max pooling + linear head + log_softmax ----
            pooled = constp.tile([P, GPC], f32)
            for k in range(GPC):
                nc.vector.reduce_max(pooled[:, k:k + 1],
                                     hT_sb[:, gb_local[k]:gb_local[k + 1]],
                                     axis=mybir.AxisListType.X)
            pl = psa.tile([GPC, N_CLASSES], f32, tag="pl")
            nc.tensor.matmul(pl[:], lhsT=pooled[:], rhs=w2_sb[:],
                             start=True, stop=True)
            ls = constp.tile([GPC, N_CLASSES], f32)
            nc.vector.tensor_add(out=ls[:], in0=pl[:], in1=b2_sb[:])
            mx = constp.tile([GPC, 1], f32)
            nc.vector.reduce_max(mx[:], ls[:], axis=mybir.AxisListType.X)
            xm = constp.tile([GPC, N_CLASSES], f32)
            nc.vector.tensor_scalar_sub(xm[:], ls[:], mx[:, 0:1])
            ex = constp.tile([GPC, N_CLASSES], f32)
            nc.scalar.activation(ex[:], xm[:],
                                 mybir.ActivationFunctionType.Exp)
            sm = constp.tile([GPC, 1], f32)
            nc.vector.reduce_sum(sm[:], ex[:], axis=mybir.AxisListType.X)
            lg = constp.tile([GPC, 1], f32)
            nc.scalar.activation(lg[:], sm[:],
                                 mybir.ActivationFunctionType.Ln)
            of = constp.tile([GPC, N_CLASSES], f32)
            nc.vector.tensor_scalar_sub(of[:], xm[:], lg[:, 0:1])
            nc.sync.dma_start(out=out.ap()[:, :], in_=of[:])

    nc.compile()
    return nc


def _install_ntff_hook():
    """Provide antenv.axon_hooks.get_axon_ntff_profile_hook when the agent
    image lacks it (ctypes bridge into libaxon_pjrt.so, mirroring
    trn_boot._ntff_profile_via_ctypes)."""
    import contextlib
    import ctypes
    import sys
    import types
    try:
        import antenv.axon_hooks  # noqa: F401
        return
    except ImportError:
        pass
    the_hook = None
    try:
        lib = ctypes.CDLL("/opt/axon/libaxon_pjrt.so")
        if hasattr(lib, "axon_start_nrt_profile"):
            lib.axon_start_nrt_profile.argtypes = [
                ctypes.POINTER(ctypes.c_int64), ctypes.c_size_t]
            lib.axon_start_nrt_profile.restype = ctypes.c_int64
            lib.axon_stop_nrt_profile.argtypes = [ctypes.c_char_p]
            lib.axon_stop_nrt_profile.restype = ctypes.c_int64

            @contextlib.contextmanager
            def _hook(output_dir, device_ids):
                import jax
                jax.devices()
                if device_ids:
                    ids = (ctypes.c_int64 * len(device_ids))(*device_ids)
                    rc = lib.axon_start_nrt_profile(ids, len(device_ids))
                else:
                    rc = lib.axon_start_nrt_profile(None, 0)
                if rc != 0:
                    raise RuntimeError(f"axon_start_nrt_profile rc={rc}")
                try:
                    yield
                finally:
                    n = lib.axon_stop_nrt_profile(str(output_dir).encode())
                    print(f"ntff profile: {n} file(s) -> {output_dir}")

            the_hook = _hook
    except OSError:
        pass
    mod = types.ModuleType("antenv.axon_hooks")
    mod.get_axon_ntff_profile_hook = lambda: the_hook
    mod.set_axon_ntff_profile_hook = lambda h: None
    import antenv
    antenv.axon_hooks = mod
    sys.modules["antenv.axon_hooks"] = mod


def kernel(x, W1, b1, W2, b2, edge_index, batch, _trace=False, _trace_kwargs=None):
    global LAST_RESULTS
    from concourse import bass_utils
    if _trace:
        _install_ntff_hook()
        # no fish/S3 in this container; keep artifacts local
        bass_utils.upload_artifacts = lambda tmpdir: tmpdir

    chunks, gb_local, in_maps = _host_prep(x, W1, b1, W2, b2,
                                           edge_index, batch)
    key = (chunks, gb_local)
    nc = _PROGRAM_CACHE.get(key)
    if nc is None:
        nc = _build_program(chunks, gb_local)
        _PROGRAM_CACHE[key] = nc

    res = bass_utils.run_bass_kernel_spmd(
        nc, in_maps, core_ids=list(range(NCORES)),
        trace=_trace, tmpdir="/tmp/gnn_neff" if _trace else None,
        **(_trace_kwargs or {}))
    LAST_RESULTS = res
    return np.concatenate([res.results[c]["out"] for c in range(NCORES)],
                          axis=0)
